# revision 16
# baseline (speedup 1.0000x reference)
"""BotRGCN forward pass on 8 Trainium2 NeuronCores (Bass/Tile SPMD).

Strategy (graph/data parallel, per sharding hint):
  - Nodes are partitioned into 128-row blocks; blocks are assigned
    round-robin-contiguously to the 8 cores (core k owns blocks
    [k*bpc, (k+1)*bpc)).
  - Feature encoder: x is transposed/cast on host to xT [1552, npc] per
    core; one fused matmul with a combined block-sparse W_enc [1552,128].
    The encoder produces h in BOTH orientations (hT for later matmuls,
    row-major h for the gather table).
  - Each RGCN layer: AllGather the row-major h table (bf16) so every core
    holds all N rows, then for each owned dst block: indirect-DMA gather
    the source rows of its edges (sorted by (dst block, relation), padded
    to 128-edge tiles on host), build a scaled one-hot selection matrix S
    [128 edges, 128 dst] on DVE (iota==dst_local)*scale with
    scale=1/max(cnt,1) precomputed on host, and matmul-accumulate
    P^T[feat,dst] = sum_tiles g^T S in PSUM.  Then
    out^T = W_r^T P_r^T + root^T h^T (+bias, LeakyReLU) — all matmuls in
    the transposed orientation so no per-block transposes are needed
    except the one producing the row-major gather table.
  - The classifier is fused into layer 2's epilogue; the final output is
    produced transposed [128, npc] and transposed back on host.

The kernel() function takes FULL inputs and returns the FULL output.
"""

import math
import sys
from contextlib import ExitStack

sys.path.insert(0, "/opt/trn_rl_repo")

import ml_dtypes
import numpy as np

import concourse.bass as bass
import concourse.bacc as bacc_mod
import concourse.tile as tile
from concourse import mybir
from concourse.bass_utils import run_bass_kernel_spmd
from concourse.masks import make_identity

NCORES = 8
P = 128
R = 2
D_IN = 1552
HID = 128
D_NUM, D_TWEET, D_CAT, D_DES = 5, 768, 11, 768

# flow dtype for activations / gather tables / matmul operands
import os as _os
if _os.environ.get("BOT_FLOW", "bf16") == "f32":
    FLOW_DT = mybir.dt.float32
    FLOW_NP = np.float32
else:
    FLOW_DT = mybir.dt.bfloat16
    FLOW_NP = ml_dtypes.bfloat16

TRACE = False            # test.py can set kernel.TRACE = True
LAST_RESULTS = None      # BassKernelResults of the last run (for test.py)
TIME_RUNS = int(_os.environ.get("BOT_TIME_RUNS", "0"))
LAST_TIME_NS = None      # min wall-time of a steady-state execute, ns
LAST_TIMES = None

F32 = mybir.dt.float32
AF = mybir.ActivationFunctionType
ALU = mybir.AluOpType


def _host_prep(x, src, dst, et, weights):
    """Build per-core xT slices, the combined encoder weight, and the
    padded per-core edge plan."""
    N = x.shape[0]
    E = src.shape[0]
    B_total = (N + P - 1) // P
    bpc = (B_total + NCORES - 1) // NCORES          # blocks per core
    npc = bpc * P                                   # nodes per core (padded)
    Npad = npc * NCORES

    # ---- combined encoder weight ----
    w_des, b_des = weights["w_des"], weights["b_des"]
    w_tweet, b_tweet = weights["w_tweet"], weights["b_tweet"]
    w_num, b_num = weights["w_num"], weights["b_num"]
    w_cat, b_cat = weights["w_cat"], weights["b_cat"]
    SUB = w_des.shape[1]
    W_enc = np.zeros((D_IN, 4 * SUB), np.float32)
    o_num, o_tweet, o_cat, o_des = 0, D_NUM, D_NUM + D_TWEET, D_NUM + D_TWEET + D_CAT
    W_enc[o_des:o_des + D_DES, 0 * SUB:1 * SUB] = w_des
    W_enc[o_tweet:o_tweet + D_TWEET, 1 * SUB:2 * SUB] = w_tweet
    W_enc[o_num:o_num + D_NUM, 2 * SUB:3 * SUB] = w_num
    W_enc[o_cat:o_cat + D_CAT, 3 * SUB:4 * SUB] = w_cat
    b_enc = np.concatenate([b_des, b_tweet, b_num, b_cat]).astype(np.float32)

    # ---- per-core xT slices (transpose + cast on host) ----
    xTs = []
    for k in range(NCORES):
        lo, hi = k * npc, min((k + 1) * npc, N)
        xk = np.zeros((D_IN, npc), FLOW_NP)
        if hi > lo:
            xk[:, : hi - lo] = x[lo:hi].T.astype(FLOW_NP)
        xTs.append(xk)

    # ---- edge plan ----
    # per-(relation, dst) counts for the mean normalization
    scale = np.zeros(E, np.float32)
    for r in range(R):
        m = et == r
        cnt_r = np.bincount(dst[m], minlength=N)
        scale[m] = 1.0 / np.maximum(cnt_r[dst[m]], 1)

    gblk = dst >> 7                                 # global dst block
    g2 = gblk * R + et                              # group id (block, rel)
    n_groups = B_total * R
    counts = np.bincount(g2, minlength=n_groups)

    # tiles per (local block, rel): max over cores so the SPMD program is
    # identical on every core
    T = np.zeros((bpc, R), np.int64)
    for lb in range(bpc):
        for r in range(R):
            gids = [(k * bpc + lb) * R + r for k in range(NCORES)]
            cmax = max(counts[g] if g < n_groups else 0 for g in gids)
            T[lb, r] = (cmax + P - 1) // P
    tile_off = np.zeros((bpc, R), np.int64)
    acc = 0
    for lb in range(bpc):
        for r in range(R):
            tile_off[lb, r] = acc
            acc += T[lb, r]
    TILES = int(acc)

    plan_src = np.zeros((NCORES, P, TILES), np.int32)
    plan_dst = np.full((NCORES, P, TILES), -1.0, np.float32)
    plan_scl = np.zeros((NCORES, P, TILES), np.float32)

    order = np.argsort(g2, kind="stable")
    g2s = g2[order]
    starts = np.zeros(n_groups + 1, np.int64)
    starts[1:] = np.cumsum(counts)
    rank = np.arange(E, dtype=np.int64) - starts[g2s]
    gb = g2s // R
    rr = g2s % R
    kk = gb // bpc
    lb = gb % bpc
    tt = tile_off[lb, rr] + (rank >> 7)
    pp = rank & 127
    plan_src[kk, pp, tt] = src[order].astype(np.int32)
    plan_dst[kk, pp, tt] = (dst[order] & 127).astype(np.float32)
    plan_scl[kk, pp, tt] = scale[order]

    return dict(
        N=N, B_total=B_total, bpc=bpc, npc=npc, Npad=Npad,
        W_enc=W_enc, b_enc=b_enc, xTs=xTs,
        T=T, tile_off=tile_off, TILES=TILES,
        plan_src=plan_src, plan_dst=plan_dst, plan_scl=plan_scl,
    )


def _build_program(prep, weights):
    bpc, npc, Npad, TILES = prep["bpc"], prep["npc"], prep["Npad"], prep["TILES"]
    T, tile_off = prep["T"], prep["tile_off"]
    KCH = (D_IN + P - 1) // P                       # 13 k-chunks (last partial)

    nc = bacc_mod.Bacc(num_devices=NCORES)

    # ---- I/O ----
    xT_t = nc.dram_tensor("xT", [D_IN, npc], FLOW_DT, kind="ExternalInput")
    psrc_t = nc.dram_tensor("plan_src", [P, TILES], mybir.dt.int32, kind="ExternalInput")
    pdst_t = nc.dram_tensor("plan_dst", [P, TILES], F32, kind="ExternalInput")
    pscl_t = nc.dram_tensor("plan_scl", [P, TILES], F32, kind="ExternalInput")
    out_t = nc.dram_tensor("outT", [P, npc], F32, kind="ExternalOutput")

    # ---- internal DRAM ----
    h_rows = [nc.dram_tensor(f"h{l}_rows", [npc, HID], FLOW_DT) for l in range(2)]
    h_full = [nc.dram_tensor(f"h{l}_full", [Npad, HID], FLOW_DT, addr_space="Shared")
              for l in range(2)]
    hT = [nc.dram_tensor(f"h{l}T", [HID, npc], FLOW_DT) for l in range(2)]

    # ---- constants (inline in NEFF, same on every core) ----
    wenc_pad = np.zeros((KCH * P, HID), FLOW_NP)
    wenc_pad[:D_IN] = prep["W_enc"].astype(FLOW_NP)
    wenc_c = nc.inline_tensor(wenc_pad, "wenc")
    benc_c = nc.inline_tensor(prep["b_enc"].reshape(HID, 1), "benc")
    win_c = nc.inline_tensor(weights["w_in"].astype(FLOW_NP), "win")
    bin_c = nc.inline_tensor(
        weights["b_in"].astype(np.float32).reshape(HID, 1), "bin")

    lw = []
    for l, (wname, rname, bname) in enumerate(
        [("rg1_w", "rg1_root", "rg1_b"), ("rg2_w", "rg2_root", "rg2_b")]
    ):
        w = weights[wname].astype(FLOW_NP)
        root = weights[rname].astype(FLOW_NP)
        b = weights[bname].astype(np.float32).reshape(HID, 1)
        lw.append(dict(
            w0=nc.inline_tensor(w[0], f"l{l}w0"),
            w1=nc.inline_tensor(w[1], f"l{l}w1"),
            root=nc.inline_tensor(root, f"l{l}root"),
            b=nc.inline_tensor(b, f"l{l}b"),
        ))
    wcls_c = nc.inline_tensor(weights["w_cls"].astype(FLOW_NP), "wcls")
    bcls_c = nc.inline_tensor(
        weights["b_cls"].astype(np.float32).reshape(HID, 1), "bcls")
    iota_c = nc.inline_tensor(
        np.tile(np.arange(P, dtype=np.float32), (P, 1)).astype(FLOW_NP), "iota")

    with ExitStack() as ctx:
        tc = ctx.enter_context(tile.TileContext(nc, num_cores=NCORES, pool_alloc_mode="queue"))
        cp = ctx.enter_context(tc.tile_pool(name="const", bufs=1))

        # persistent constants in SBUF
        wenc_sb = cp.tile([P, KCH * P], FLOW_DT)
        for k in range(KCH):
            pk = min(P, D_IN - k * P)
            nc.sync.dma_start(out=wenc_sb[:pk, k * P:(k + 1) * P],
                              in_=wenc_c[k * P:k * P + pk, :])
        benc_sb = cp.tile([P, 1], F32)
        nc.sync.dma_start(out=benc_sb[:], in_=benc_c[:, :])
        win_sb = cp.tile([P, P], FLOW_DT)
        nc.sync.dma_start(out=win_sb[:], in_=win_c[:, :])
        bin_sb = cp.tile([P, 1], F32)
        nc.sync.dma_start(out=bin_sb[:], in_=bin_c[:, :])
        lsb = []
        for l in range(2):
            d = {}
            for key in ("w0", "w1", "root"):
                t_ = cp.tile([P, P], FLOW_DT, tag=f"w_l{l}_{key}")
                nc.sync.dma_start(out=t_[:], in_=lw[l][key][:, :])
                d[key] = t_
            bt = cp.tile([P, 1], F32, tag=f"b_l{l}")
            nc.sync.dma_start(out=bt[:], in_=lw[l]["b"][:, :])
            d["b"] = bt
            lsb.append(d)
        wcls_sb = cp.tile([P, P], FLOW_DT)
        nc.sync.dma_start(out=wcls_sb[:], in_=wcls_c[:, :])
        bcls_sb = cp.tile([P, 1], F32)
        nc.sync.dma_start(out=bcls_sb[:], in_=bcls_c[:, :])
        iota_sb = cp.tile([P, P], FLOW_DT)
        nc.sync.dma_start(out=iota_sb[:], in_=iota_c[:, :])
        ident_sb = cp.tile([P, P], FLOW_DT)
        make_identity(nc, ident_sb[:])

        psrc_sb = cp.tile([P, TILES], mybir.dt.int32)
        nc.sync.dma_start(out=psrc_sb[:], in_=psrc_t[:, :])
        pdst_sb = cp.tile([P, TILES], F32)
        nc.sync.dma_start(out=pdst_sb[:], in_=pdst_t[:, :])
        pscl_sb = cp.tile([P, TILES], F32)
        nc.sync.dma_start(out=pscl_sb[:], in_=pscl_t[:, :])

        # ================= encoder =================
        with (
            tc.tile_pool(name="enc_sb", bufs=2) as ep,
            tc.tile_pool(name="enc_out", bufs=2) as hp_pool,
            tc.tile_pool(name="enc_rows", bufs=3) as rp,
            tc.tile_pool(name="enc_ps", bufs=2, space="PSUM") as pp,
        ):
            SW = 512
            off = 0
            while off < npc:
                w = min(SW, npc - off)
                hpsum = pp.tile([P, SW], F32, tag="enc")
                xall = ep.tile([P, KCH * SW], FLOW_DT, tag="xall")
                for k in range(KCH):
                    pk = min(P, D_IN - k * P)
                    nc.sync.dma_start(out=xall[:pk, k * SW:k * SW + w],
                                      in_=xT_t[k * P:k * P + pk, off:off + w])
                for k in range(KCH):
                    pk = min(P, D_IN - k * P)
                    nc.tensor.matmul(out=hpsum[:, :w],
                                     lhsT=wenc_sb[:pk, k * P:(k + 1) * P],
                                     rhs=xall[:pk, k * SW:k * SW + w],
                                     start=(k == 0), stop=(k == KCH - 1))
                hs = hp_pool.tile([P, SW], FLOW_DT, tag="henc")
                nc.scalar.activation(out=hs[:, :w], in_=hpsum[:, :w],
                                     func=AF.Lrelu, bias=benc_sb[:, :1],
                                     alpha=0.01)
                # second encoder linear: h = leaky(w_in.T @ h + b_in)
                h2psum = pp.tile([P, SW], F32, tag="enc2")
                nc.tensor.matmul(out=h2psum[:, :w], lhsT=win_sb[:],
                                 rhs=hs[:, :w], start=True, stop=True)
                hs2 = hp_pool.tile([P, SW], FLOW_DT, tag="henc2")
                nc.scalar.activation(out=hs2[:, :w], in_=h2psum[:, :w],
                                     func=AF.Lrelu, bias=bin_sb[:, :1],
                                     alpha=0.01)
                nc.sync.dma_start(out=hT[0][:, off:off + w], in_=hs2[:, :w])
                for t in range(w // P):
                    tp = pp.tile([P, P], FLOW_DT, tag="tr")
                    nc.tensor.transpose(out=tp[:], in_=hs2[:, t * P:(t + 1) * P],
                                        identity=ident_sb[:])
                    rs = rp.tile([P, P], FLOW_DT, tag="rows")
                    nc.vector.tensor_copy(out=rs[:], in_=tp[:])
                    nc.sync.dma_start(
                        out=h_rows[0][off + t * P:off + (t + 1) * P, :], in_=rs[:])
                off += w

        nc.gpsimd.collective_compute(
            "AllGather", ALU.bypass, replica_groups=[list(range(NCORES))],
            ins=[h_rows[0][:, :]], outs=[h_full[0][:, :]])
        tc.strict_bb_all_engine_barrier()

        # ================= RGCN layers =================
        Tmax = int(T.max()) if T.size else 1
        for l in range(2):
            with (
                tc.tile_pool(name=f"l{l}_sb", bufs=6) as gp,
                tc.tile_pool(name=f"l{l}_s", bufs=6) as sp,
                tc.tile_pool(name=f"l{l}_misc", bufs=6) as mp,
                tc.tile_pool(name=f"l{l}_ps", bufs=2, space="PSUM") as pp,
            ):
                for b in range(bpc):
                    hTb = mp.tile([P, P], FLOW_DT, tag="hTb")
                    nc.sync.dma_start(out=hTb[:],
                                      in_=hT[l][:, b * P:(b + 1) * P])
                    Ps = [None, None]
                    for r in range(R):
                        Tbr = int(T[b, r])
                        if Tbr == 0:
                            continue
                        t0 = int(tile_off[b, r])
                        g = gp.tile([P, Tbr * P], FLOW_DT, tag=f"g{Tbr}")
                        for t in range(Tbr):
                            nc.gpsimd.indirect_dma_start(
                                out=g[:, t * P:(t + 1) * P], out_offset=None,
                                in_=h_full[l][:, :],
                                in_offset=bass.IndirectOffsetOnAxis(
                                    ap=psrc_sb[:, t0 + t:t0 + t + 1], axis=0))
                        Pp = pp.tile([P, P], F32, tag=f"P{r}")
                        for t in range(Tbr):
                            S = sp.tile([P, P], FLOW_DT, tag="s")
                            nc.vector.tensor_scalar(
                                out=S[:], in0=iota_sb[:],
                                scalar1=pdst_sb[:, t0 + t:t0 + t + 1],
                                scalar2=pscl_sb[:, t0 + t:t0 + t + 1],
                                op0=ALU.is_equal, op1=ALU.mult)
                            nc.tensor.matmul(out=Pp[:],
                                             lhsT=g[:, t * P:(t + 1) * P],
                                             rhs=S[:],
                                             start=(t == 0), stop=(t == Tbr - 1))
                        Psr = mp.tile([P, P], FLOW_DT, tag=f"ps{r}")
                        nc.vector.tensor_copy(out=Psr[:], in_=Pp[:])
                        Ps[r] = Psr

                    op_ = pp.tile([P, P], F32, tag="out")
                    mms = []
                    if Ps[0] is not None:
                        mms.append((lsb[l]["w0"], Ps[0]))
                    if Ps[1] is not None:
                        mms.append((lsb[l]["w1"], Ps[1]))
                    mms.append((lsb[l]["root"], hTb))
                    for i, (lhsT, rhs) in enumerate(mms):
                        nc.tensor.matmul(out=op_[:], lhsT=lhsT[:], rhs=rhs[:],
                                         start=(i == 0), stop=(i == len(mms) - 1))
                    ho = mp.tile([P, P], FLOW_DT, tag="ho")
                    nc.scalar.activation(out=ho[:], in_=op_[:], func=AF.Lrelu,
                                         bias=lsb[l]["b"][:, :1], alpha=0.01)
                    if l == 0:
                        nc.sync.dma_start(out=hT[1][:, b * P:(b + 1) * P],
                                          in_=ho[:])
                        tp = pp.tile([P, P], FLOW_DT, tag="tr2")
                        nc.tensor.transpose(out=tp[:], in_=ho[:],
                                            identity=ident_sb[:])
                        rs = mp.tile([P, P], FLOW_DT, tag="rows")
                        nc.vector.tensor_copy(out=rs[:], in_=tp[:])
                        nc.sync.dma_start(
                            out=h_rows[1][b * P:(b + 1) * P, :], in_=rs[:])
                    else:
                        cpsum = pp.tile([P, P], F32, tag="tr2")
                        nc.tensor.matmul(out=cpsum[:], lhsT=wcls_sb[:],
                                         rhs=ho[:], start=True, stop=True)
                        osb = mp.tile([P, P], F32, tag="osb")
                        nc.vector.tensor_scalar(
                            out=osb[:], in0=cpsum[:], scalar1=bcls_sb[:, :1],
                            scalar2=None, op0=ALU.add)
                        nc.sync.dma_start(out=out_t[:, b * P:(b + 1) * P],
                                          in_=osb[:])

            if l == 0:
                nc.gpsimd.collective_compute(
                    "AllGather", ALU.bypass,
                    replica_groups=[list(range(NCORES))],
                    ins=[h_rows[1][:, :]], outs=[h_full[1][:, :]])
                tc.strict_bb_all_engine_barrier()

    if not nc.is_finalized():
        nc.finalize()
    return nc


def kernel(**inputs):
    global LAST_RESULTS
    x = np.asarray(inputs["x"], np.float32)
    ei = np.asarray(inputs["edge_index"])
    et = np.asarray(inputs["edge_type"]).astype(np.int64)
    src = ei[0].astype(np.int64)
    dst = ei[1].astype(np.int64)

    weights = {k: np.asarray(v, np.float32) for k, v in inputs.items()
               if k not in ("x", "edge_index", "edge_type")}

    prep = _host_prep(x, src, dst, et, weights)
    nc = _build_program(prep, weights)

    in_maps = []
    for k in range(NCORES):
        in_maps.append({
            "xT": prep["xTs"][k],
            "plan_src": prep["plan_src"][k],
            "plan_dst": prep["plan_dst"][k],
            "plan_scl": prep["plan_scl"][k],
        })

    if TIME_RUNS > 0:
        results = _run_and_time(nc, in_maps, TIME_RUNS)
    else:
        res = run_bass_kernel_spmd(nc, in_maps, list(range(NCORES)), trace=TRACE)
        LAST_RESULTS = res
        results = res.results

    outs = [results[k]["outT"].T for k in range(NCORES)]
    out = np.concatenate(outs, axis=0)[: prep["N"]]
    return np.ascontiguousarray(out, dtype=np.float32)


def _run_and_time(nc, in_maps, n_runs):
    """Mirror bass2jax.run_bass_via_pjrt's multi-core path, but jit once,
    pre-place inputs on the device mesh, and wall-clock repeated executes."""
    global LAST_TIME_NS, LAST_TIMES
    import time as _time
    import jax
    from jax.sharding import Mesh, PartitionSpec, NamedSharding
    from jax.experimental.shard_map import shard_map
    from concourse import bass2jax, mybir as _mb
    bass2jax.install_neuronx_cc_hook()

    partition_name = nc.partition_id_tensor.name if nc.partition_id_tensor else None
    in_names, out_names, out_avals, zero_outs = [], [], [], []
    for alloc in nc.m.functions[0].allocations:
        if not isinstance(alloc, _mb.MemoryLocationSet):
            continue
        name = alloc.memorylocations[0].name
        if alloc.kind == "ExternalInput":
            if name != partition_name:
                in_names.append(name)
        elif alloc.kind == "ExternalOutput":
            shape = tuple(alloc.tensor_shape)
            dtype = _mb.dt.np(alloc.dtype)
            out_names.append(name)
            out_avals.append(jax.core.ShapedArray(shape, dtype))
            zero_outs.append(np.zeros(shape, dtype))
    n_params = len(in_names)
    in_names = in_names + out_names
    if partition_name is not None:
        in_names.append(partition_name)

    def _body(*args):
        operands = list(args)
        if partition_name is not None:
            operands.append(bass2jax.partition_id_tensor())
        outs = bass2jax._bass_exec_p.bind(
            *operands,
            out_avals=tuple(out_avals),
            in_names=tuple(in_names),
            out_names=tuple(out_names),
            lowering_input_output_aliases=(),
            sim_require_finite=True,
            sim_require_nnan=True,
            nc=nc,
        )
        return tuple(outs)

    devices = jax.devices()[:NCORES]
    mesh = Mesh(np.asarray(devices), ("core",))
    n_outs = len(out_names)
    in_specs = (PartitionSpec("core"),) * (n_params + n_outs)
    out_specs = (PartitionSpec("core"),) * n_outs
    sharded = jax.jit(
        shard_map(_body, mesh=mesh, in_specs=in_specs, out_specs=out_specs,
                  check_rep=False),
        keep_unused=True,
    )
    per_core = [[np.asarray(m[name]) for name in in_names[:n_params]]
                for m in in_maps]
    sh = NamedSharding(mesh, PartitionSpec("core"))
    concat_in = [
        jax.device_put(
            np.concatenate([per_core[c][i] for c in range(NCORES)], axis=0), sh)
        for i in range(n_params)
    ]
    concat_zeros = [
        jax.device_put(np.zeros((NCORES * z.shape[0], *z.shape[1:]), z.dtype), sh)
        for z in zero_outs
    ]
    jax.block_until_ready(concat_in)
    jax.block_until_ready(concat_zeros)

    times = []
    out_arrs = None
    for i in range(max(2, n_runs)):
        t0 = _time.perf_counter()
        out_arrs = sharded(*concat_in, *concat_zeros)
        jax.block_until_ready(out_arrs)
        times.append(_time.perf_counter() - t0)
    LAST_TIMES = times
    LAST_TIME_NS = int(min(times[1:]) * 1e9)   # skip first (compile)
    return [
        {name: np.asarray(out_arrs[i]).reshape(NCORES, *out_avals[i].shape)[c]
         for i, name in enumerate(out_names)}
        for c in range(NCORES)
    ]


# revision 19
# speedup vs baseline: 9.7189x; 9.7189x over previous
"""BotRGCN forward pass on 8 Trainium2 NeuronCores (Bass/Tile SPMD).

Strategy (graph/data parallel, per the sharding hint):
  - Nodes are split into 128-row blocks; core k owns blocks
    [k*bpc, (k+1)*bpc).
  - Feature encoder: x is transposed/cast to bf16 on host (xT [1552, npc]
    per core); one fused matmul against a combined block-sparse
    W_enc [1552,128], then the w_in linear.  The encoder emits h in both
    orientations (hT for later matmuls, row-major h for the gather table).
  - Per RGCN layer: AllGather the row-major bf16 h table so every core
    holds all N rows; for each owned 128-dst block, indirect-DMA gather
    the source rows of its edges (host-sorted by dst block, padded to
    128-edge tiles), build a scaled selection matrix
    S2 [128 edges, 256] = (iota256 == dst_local + 128*rel) * (1/cnt)
    on DVE, and matmul-accumulate both relations' partial sums
    P^T [feat, 2*128 dst] in PSUM.  Blocks are processed 3 at a time so
    the transform/root matmuls run 384 wide and DMAs are batched.
    out^T = W_r^T P_r^T + root^T h^T (+bias, LeakyReLU) stays in the
    transposed orientation; only the gather-table rows need an on-chip
    transpose.  The classifier is fused into layer 2; the final output is
    produced transposed [128, npc] and transposed back on host.

kernel() takes FULL inputs and returns the FULL output.
"""

import math
import os as _os
import sys
from contextlib import ExitStack

sys.path.insert(0, "/opt/trn_rl_repo")

import ml_dtypes
import numpy as np

import concourse.bass as bass
import concourse.bacc as bacc_mod
import concourse.tile as tile
from concourse import mybir
from concourse.bass_utils import run_bass_kernel_spmd
from concourse.masks import make_identity

NCORES = 8
P = 128
R = 2
SB = 3                   # dst blocks per super-block (PSUM budget: 8 banks)
D_IN = 1552
HID = 128
D_NUM, D_TWEET, D_CAT, D_DES = 5, 768, 11, 768

# flow dtype for activations / gather tables / matmul operands
if _os.environ.get("BOT_FLOW", "bf16") == "f32":
    FLOW_DT = mybir.dt.float32
    FLOW_NP = np.float32
else:
    FLOW_DT = mybir.dt.bfloat16
    FLOW_NP = ml_dtypes.bfloat16

TRACE = False
LAST_RESULTS = None
TIME_RUNS = int(_os.environ.get("BOT_TIME_RUNS", "0"))
LAST_TIME_NS = None
LAST_TIMES = None

F32 = mybir.dt.float32
AF = mybir.ActivationFunctionType
ALU = mybir.AluOpType


def _host_prep(x, src, dst, et, weights):
    N = x.shape[0]
    E = src.shape[0]
    B_total = (N + P - 1) // P
    bpc = (B_total + NCORES - 1) // NCORES
    bpc = SB * ((bpc + SB - 1) // SB)               # multiple of SB
    npc = bpc * P
    Npad = npc * NCORES

    # ---- combined encoder weight ----
    w_des, b_des = weights["w_des"], weights["b_des"]
    w_tweet, b_tweet = weights["w_tweet"], weights["b_tweet"]
    w_num, b_num = weights["w_num"], weights["b_num"]
    w_cat, b_cat = weights["w_cat"], weights["b_cat"]
    SUB = w_des.shape[1]
    W_enc = np.zeros((D_IN, 4 * SUB), np.float32)
    o_num, o_tweet, o_cat, o_des = 0, D_NUM, D_NUM + D_TWEET, D_NUM + D_TWEET + D_CAT
    W_enc[o_des:o_des + D_DES, 0 * SUB:1 * SUB] = w_des
    W_enc[o_tweet:o_tweet + D_TWEET, 1 * SUB:2 * SUB] = w_tweet
    W_enc[o_num:o_num + D_NUM, 2 * SUB:3 * SUB] = w_num
    W_enc[o_cat:o_cat + D_CAT, 3 * SUB:4 * SUB] = w_cat
    b_enc = np.concatenate([b_des, b_tweet, b_num, b_cat]).astype(np.float32)

    # ---- per-core xT slices ----
    xTs = []
    for k in range(NCORES):
        lo, hi = k * npc, min((k + 1) * npc, N)
        xk = np.zeros((D_IN, npc), FLOW_NP)
        if hi > lo:
            xk[:, : hi - lo] = x[lo:hi].T.astype(FLOW_NP)
        xTs.append(xk)

    # ---- edge plan: group by dst block, both relations together ----
    scale = np.zeros(E, np.float32)
    for r in range(R):
        m = et == r
        cnt_r = np.bincount(dst[m], minlength=N)
        scale[m] = 1.0 / np.maximum(cnt_r[dst[m]], 1)

    gblk = (dst >> 7).astype(np.int64)              # global dst block
    counts = np.bincount(gblk, minlength=B_total)

    # tiles per local block: max over cores (identical SPMD program)
    T = np.zeros(bpc, np.int64)
    for lb in range(bpc):
        cmax = 0
        for k in range(NCORES):
            g = k * bpc + lb
            if g < B_total:
                cmax = max(cmax, counts[g])
        T[lb] = max(1, (cmax + P - 1) // P)
    tile_off = np.zeros(bpc, np.int64)
    tile_off[1:] = np.cumsum(T)[:-1]
    TILES = int(T.sum())

    plan_src = np.zeros((NCORES, P, TILES), np.int32)
    plan_dst = np.full((NCORES, P, TILES), -1.0, np.float32)
    plan_scl = np.zeros((NCORES, P, TILES), np.float32)

    order = np.argsort(gblk, kind="stable")
    gs = gblk[order]
    starts = np.zeros(B_total + 1, np.int64)
    starts[1:] = np.cumsum(counts)
    rank = np.arange(E, dtype=np.int64) - starts[gs]
    kk = gs // bpc
    lb = gs % bpc
    tt = tile_off[lb] + (rank >> 7)
    pp = rank & 127
    plan_src[kk, pp, tt] = src[order].astype(np.int32)
    plan_dst[kk, pp, tt] = ((dst[order] & 127) + P * et[order]).astype(np.float32)
    plan_scl[kk, pp, tt] = scale[order]

    return dict(
        N=N, B_total=B_total, bpc=bpc, npc=npc, Npad=Npad,
        W_enc=W_enc, b_enc=b_enc, xTs=xTs,
        T=T, tile_off=tile_off, TILES=TILES,
        plan_src=plan_src, plan_dst=plan_dst, plan_scl=plan_scl,
    )


def _build_program(prep, weights):
    bpc, npc, Npad, TILES = prep["bpc"], prep["npc"], prep["Npad"], prep["TILES"]
    T, tile_off = prep["T"], prep["tile_off"]
    KCH = (D_IN + P - 1) // P                       # 13 chunks; last is 16 rows
    KFULL = D_IN // P                               # 12 full chunks

    nc = bacc_mod.Bacc(num_devices=NCORES)

    # ---- I/O ----
    xT_t = nc.dram_tensor("xT", [D_IN, npc], FLOW_DT, kind="ExternalInput")
    psrc_t = nc.dram_tensor("plan_src", [P, TILES], mybir.dt.int32, kind="ExternalInput")
    pdst_t = nc.dram_tensor("plan_dst", [P, TILES], F32, kind="ExternalInput")
    pscl_t = nc.dram_tensor("plan_scl", [P, TILES], F32, kind="ExternalInput")
    out_t = nc.dram_tensor("outT", [P, npc], F32, kind="ExternalOutput")

    # ---- internal DRAM ----
    h_rows = [nc.dram_tensor(f"h{l}_rows", [npc, HID], FLOW_DT) for l in range(2)]
    h_full = [nc.dram_tensor(f"h{l}_full", [Npad, HID], FLOW_DT, addr_space="Shared")
              for l in range(2)]
    hT = [nc.dram_tensor(f"h{l}T", [HID, npc], FLOW_DT) for l in range(2)]

    # ---- constants ----
    wenc_pad = np.zeros((KCH * P, HID), FLOW_NP)
    wenc_pad[:D_IN] = prep["W_enc"].astype(FLOW_NP)
    wenc_c = nc.inline_tensor(wenc_pad, "wenc")
    benc_c = nc.inline_tensor(prep["b_enc"].reshape(HID, 1), "benc")
    win_c = nc.inline_tensor(weights["w_in"].astype(FLOW_NP), "win")
    bin_c = nc.inline_tensor(
        weights["b_in"].astype(np.float32).reshape(HID, 1), "bin")

    lw = []
    for l, (wname, rname, bname) in enumerate(
        [("rg1_w", "rg1_root", "rg1_b"), ("rg2_w", "rg2_root", "rg2_b")]
    ):
        w = weights[wname].astype(FLOW_NP)
        root = weights[rname].astype(FLOW_NP)
        b = weights[bname].astype(np.float32).reshape(HID, 1)
        lw.append(dict(
            w0=nc.inline_tensor(w[0], f"l{l}w0"),
            w1=nc.inline_tensor(w[1], f"l{l}w1"),
            root=nc.inline_tensor(root, f"l{l}root"),
            b=nc.inline_tensor(b, f"l{l}b"),
        ))
    wcls_c = nc.inline_tensor(weights["w_cls"].astype(FLOW_NP), "wcls")
    bcls_c = nc.inline_tensor(
        weights["b_cls"].astype(np.float32).reshape(HID, 1), "bcls")
    iota2_c = nc.inline_tensor(
        np.tile(np.arange(2 * P, dtype=np.float32), (P, 1)).astype(FLOW_NP),
        "iota2")

    with ExitStack() as ctx:
        tc = ctx.enter_context(tile.TileContext(
            nc, num_cores=NCORES, pool_alloc_mode="queue",
            trace_sim=bool(int(_os.environ.get("BOT_TRACE_SIM", "0")))))
        cp = ctx.enter_context(tc.tile_pool(name="const", bufs=1))

        wenc_sb = cp.tile([P, KCH * P], FLOW_DT)
        for k in range(KCH):
            pk = min(P, D_IN - k * P)
            nc.sync.dma_start(out=wenc_sb[:pk, k * P:(k + 1) * P],
                              in_=wenc_c[k * P:k * P + pk, :])
        benc_sb = cp.tile([P, 1], F32)
        nc.sync.dma_start(out=benc_sb[:], in_=benc_c[:, :])
        win_sb = cp.tile([P, P], FLOW_DT)
        nc.sync.dma_start(out=win_sb[:], in_=win_c[:, :])
        bin_sb = cp.tile([P, 1], F32)
        nc.sync.dma_start(out=bin_sb[:], in_=bin_c[:, :])
        lsb = []
        for l in range(2):
            d = {}
            for key in ("w0", "w1", "root"):
                t_ = cp.tile([P, P], FLOW_DT, tag=f"w_l{l}_{key}")
                nc.sync.dma_start(out=t_[:], in_=lw[l][key][:, :])
                d[key] = t_
            bt = cp.tile([P, 1], F32, tag=f"b_l{l}")
            nc.sync.dma_start(out=bt[:], in_=lw[l]["b"][:, :])
            d["b"] = bt
            lsb.append(d)
        wcls_sb = cp.tile([P, P], FLOW_DT)
        nc.sync.dma_start(out=wcls_sb[:], in_=wcls_c[:, :])
        bcls_sb = cp.tile([P, 1], F32)
        nc.sync.dma_start(out=bcls_sb[:], in_=bcls_c[:, :])
        iota2_sb = cp.tile([P, 2 * P], FLOW_DT)
        nc.sync.dma_start(out=iota2_sb[:], in_=iota2_c[:, :])
        ident_sb = cp.tile([P, P], FLOW_DT)
        make_identity(nc, ident_sb[:])

        psrc_sb = cp.tile([P, TILES], mybir.dt.int32)
        nc.sync.dma_start(out=psrc_sb[:], in_=psrc_t[:, :])
        pdst_sb = cp.tile([P, TILES], F32)
        nc.sync.dma_start(out=pdst_sb[:], in_=pdst_t[:, :])
        pscl_sb = cp.tile([P, TILES], F32)
        nc.sync.dma_start(out=pscl_sb[:], in_=pscl_t[:, :])

        REPEAT = int(_os.environ.get("BOT_REPEAT", "1"))
        SKIP = set(_os.environ.get("BOT_SKIP", "").split(","))
        for _rep in range(REPEAT):
            if _rep > 0:
                tc.strict_bb_all_engine_barrier()

            # ================= encoder =================
            with (
                tc.tile_pool(name="enc_sb", bufs=2) as ep,
                tc.tile_pool(name="enc_out", bufs=2) as hp_pool,
                tc.tile_pool(name="enc_rows", bufs=3) as rp,
                tc.tile_pool(name="enc_ps", bufs=2, space="PSUM") as pp,
            ):
                SW = 512
                off = 0
                while off < npc:
                    w = min(SW, npc - off)
                    nb = w // P
                    hpsum = pp.tile([P, SW], F32, tag="enc")
                    xall = ep.tile([P, KCH * SW], FLOW_DT, tag="xall")
                    # batched load: 12 full chunks in one DMA + the 16-row tail
                    nc.sync.dma_start(
                        out=xall[:, :].rearrange(
                            "p (k n) -> p k n", n=SW)[:, :KFULL, :w],
                        in_=xT_t[0:KFULL * P, off:off + w].rearrange(
                            "(k p) n -> p k n", p=P))
                    nc.sync.dma_start(
                        out=xall[:D_IN - KFULL * P,
                                 KFULL * SW:KFULL * SW + w],
                        in_=xT_t[KFULL * P:D_IN, off:off + w])
                    for k in range(KCH):
                        pk = min(P, D_IN - k * P)
                        nc.tensor.matmul(out=hpsum[:, :w],
                                         lhsT=wenc_sb[:pk, k * P:(k + 1) * P],
                                         rhs=xall[:pk, k * SW:k * SW + w],
                                         start=(k == 0), stop=(k == KCH - 1))
                    hs = hp_pool.tile([P, SW], FLOW_DT, tag="henc")
                    nc.scalar.activation(out=hs[:, :w], in_=hpsum[:, :w],
                                         func=AF.Lrelu, bias=benc_sb[:, :1],
                                         alpha=0.01)
                    h2psum = pp.tile([P, SW], F32, tag="enc2")
                    nc.tensor.matmul(out=h2psum[:, :w], lhsT=win_sb[:],
                                     rhs=hs[:, :w], start=True, stop=True)
                    hs2 = hp_pool.tile([P, SW], FLOW_DT, tag="henc2")
                    nc.scalar.activation(out=hs2[:, :w], in_=h2psum[:, :w],
                                         func=AF.Lrelu, bias=bin_sb[:, :1],
                                         alpha=0.01)
                    nc.sync.dma_start(out=hT[0][:, off:off + w], in_=hs2[:, :w])
                    rows = rp.tile([P, SW], FLOW_DT, tag="rows")
                    for t in range(nb):
                        tp = pp.tile([P, P], FLOW_DT, tag="tr")
                        nc.tensor.transpose(out=tp[:],
                                            in_=hs2[:, t * P:(t + 1) * P],
                                            identity=ident_sb[:])
                        nc.vector.tensor_copy(out=rows[:, t * P:(t + 1) * P],
                                              in_=tp[:])
                    nc.sync.dma_start(
                        out=h_rows[0][off:off + w, :].rearrange(
                            "(b p) f -> p b f", p=P),
                        in_=rows[:, :w].rearrange("p (b f) -> p b f", f=P))
                    off += w

            if "ag" not in SKIP:
                nc.gpsimd.collective_compute(
                    "AllGather", ALU.bypass, replica_groups=[list(range(NCORES))],
                    ins=[h_rows[0][:, :]], outs=[h_full[0][:, :]])
            tc.strict_bb_all_engine_barrier()

            # ================= RGCN layers =================
            for l in ([], range(2))["layers" not in SKIP]:
                with (
                    tc.tile_pool(name=f"l{l}_g", bufs=6) as gp,
                    tc.tile_pool(name=f"l{l}_s", bufs=8) as sp,
                    tc.tile_pool(name=f"l{l}_m", bufs=4) as mp,
                    tc.tile_pool(name=f"l{l}_ps", bufs=2, space="PSUM") as pp,
                ):
                    for b0 in range(0, bpc, SB):
                        hTb = mp.tile([P, SB * P], FLOW_DT, tag="hTb")
                        nc.sync.dma_start(
                            out=hTb[:],
                            in_=hT[l][:, b0 * P:(b0 + SB) * P])
                        # edge aggregation: P3^T [feat, SB*2*128] in PSUM
                        P3 = pp.tile([P, SB * 2 * P], F32, tag="P3")
                        for bi in range(SB):
                            b = b0 + bi
                            Tb = int(T[b])
                            t0 = int(tile_off[b])
                            g = gp.tile([P, Tb * P], FLOW_DT, tag=f"g{Tb}")
                            if "gathers" not in SKIP:
                                for t in range(Tb):
                                    nc.gpsimd.indirect_dma_start(
                                        out=g[:, t * P:(t + 1) * P],
                                        out_offset=None,
                                        in_=h_full[l][:, :],
                                        in_offset=bass.IndirectOffsetOnAxis(
                                            ap=psrc_sb[:, t0 + t:t0 + t + 1],
                                            axis=0))
                            for t in range(Tb):
                                S = sp.tile([P, 2 * P], FLOW_DT, tag="s")
                                nc.vector.tensor_scalar(
                                    out=S[:], in0=iota2_sb[:],
                                    scalar1=pdst_sb[:, t0 + t:t0 + t + 1],
                                    scalar2=pscl_sb[:, t0 + t:t0 + t + 1],
                                    op0=ALU.is_equal, op1=ALU.mult)
                                nc.tensor.matmul(
                                    out=P3[:, bi * 2 * P:(bi + 1) * 2 * P],
                                    lhsT=g[:, t * P:(t + 1) * P],
                                    rhs=S[:],
                                    start=(t == 0), stop=(t == Tb - 1))
                        Ps = mp.tile([P, SB * 2 * P], FLOW_DT, tag="ps")
                        nc.vector.tensor_copy(out=Ps[:], in_=P3[:])

                        op_ = pp.tile([P, SB * P], F32, tag="out")
                        for r in range(R):
                            rhs = Ps[:, :].rearrange(
                                "p (b c) -> p b c", c=2 * P)[:, :, r * P:(r + 1) * P]
                            nc.tensor.matmul(out=op_[:],
                                             lhsT=lsb[l][f"w{r}"][:], rhs=rhs,
                                             start=(r == 0), stop=False)
                        nc.tensor.matmul(out=op_[:], lhsT=lsb[l]["root"][:],
                                         rhs=hTb[:], start=False, stop=True)
                        ho = mp.tile([P, SB * P], FLOW_DT, tag="ho")
                        nc.scalar.activation(out=ho[:], in_=op_[:],
                                             func=AF.Lrelu,
                                             bias=lsb[l]["b"][:, :1], alpha=0.01)
                        if l == 0:
                            nc.sync.dma_start(
                                out=hT[1][:, b0 * P:(b0 + SB) * P], in_=ho[:])
                            rows = mp.tile([P, SB * P], FLOW_DT, tag="rows")
                            for t in range(SB):
                                tp = pp.tile([P, P], FLOW_DT, tag="tr2")
                                nc.tensor.transpose(
                                    out=tp[:], in_=ho[:, t * P:(t + 1) * P],
                                    identity=ident_sb[:])
                                nc.vector.tensor_copy(
                                    out=rows[:, t * P:(t + 1) * P], in_=tp[:])
                            nc.sync.dma_start(
                                out=h_rows[1][b0 * P:(b0 + SB) * P, :].rearrange(
                                    "(b p) f -> p b f", p=P),
                                in_=rows[:, :].rearrange("p (b f) -> p b f", f=P))
                        else:
                            cpsum = pp.tile([P, SB * P], F32, tag="tr2")
                            nc.tensor.matmul(out=cpsum[:], lhsT=wcls_sb[:],
                                             rhs=ho[:], start=True, stop=True)
                            osb = mp.tile([P, SB * P], F32, tag="osb")
                            nc.vector.tensor_scalar(
                                out=osb[:], in0=cpsum[:], scalar1=bcls_sb[:, :1],
                                scalar2=None, op0=ALU.add)
                            nc.sync.dma_start(
                                out=out_t[:, b0 * P:(b0 + SB) * P], in_=osb[:])

                if l == 0:
                    if "ag" not in SKIP:
                        nc.gpsimd.collective_compute(
                            "AllGather", ALU.bypass,
                            replica_groups=[list(range(NCORES))],
                            ins=[h_rows[1][:, :]], outs=[h_full[1][:, :]])
                    tc.strict_bb_all_engine_barrier()

    if not nc.is_finalized():
        nc.finalize()
    return nc


def kernel(**inputs):
    global LAST_RESULTS
    x = np.asarray(inputs["x"], np.float32)
    ei = np.asarray(inputs["edge_index"])
    et = np.asarray(inputs["edge_type"]).astype(np.int64)
    src = ei[0].astype(np.int64)
    dst = ei[1].astype(np.int64)

    weights = {k: np.asarray(v, np.float32) for k, v in inputs.items()
               if k not in ("x", "edge_index", "edge_type")}

    prep = _host_prep(x, src, dst, et, weights)
    nc = _build_program(prep, weights)

    in_maps = []
    for k in range(NCORES):
        in_maps.append({
            "xT": prep["xTs"][k],
            "plan_src": prep["plan_src"][k],
            "plan_dst": prep["plan_dst"][k],
            "plan_scl": prep["plan_scl"][k],
        })

    if TIME_RUNS > 0:
        results = _run_and_time(nc, in_maps, TIME_RUNS)
    else:
        res = run_bass_kernel_spmd(nc, in_maps, list(range(NCORES)), trace=TRACE)
        LAST_RESULTS = res
        results = res.results

    outs = [results[k]["outT"].T for k in range(NCORES)]
    out = np.concatenate(outs, axis=0)[: prep["N"]]
    return np.ascontiguousarray(out, dtype=np.float32)


def _run_and_time(nc, in_maps, n_runs):
    """Mirror bass2jax.run_bass_via_pjrt's multi-core path, but jit once,
    pre-place inputs on the device mesh, and wall-clock repeated executes."""
    global LAST_TIME_NS, LAST_TIMES
    import time as _time
    import jax
    from jax.sharding import Mesh, PartitionSpec, NamedSharding
    from jax.experimental.shard_map import shard_map
    from concourse import bass2jax, mybir as _mb
    bass2jax.install_neuronx_cc_hook()

    partition_name = nc.partition_id_tensor.name if nc.partition_id_tensor else None
    in_names, out_names, out_avals, zero_outs = [], [], [], []
    for alloc in nc.m.functions[0].allocations:
        if not isinstance(alloc, _mb.MemoryLocationSet):
            continue
        name = alloc.memorylocations[0].name
        if alloc.kind == "ExternalInput":
            if name != partition_name:
                in_names.append(name)
        elif alloc.kind == "ExternalOutput":
            shape = tuple(alloc.tensor_shape)
            dtype = _mb.dt.np(alloc.dtype)
            out_names.append(name)
            out_avals.append(jax.core.ShapedArray(shape, dtype))
            zero_outs.append(np.zeros(shape, dtype))
    n_params = len(in_names)
    in_names = in_names + out_names
    if partition_name is not None:
        in_names.append(partition_name)

    def _body(*args):
        operands = list(args)
        if partition_name is not None:
            operands.append(bass2jax.partition_id_tensor())
        outs = bass2jax._bass_exec_p.bind(
            *operands,
            out_avals=tuple(out_avals),
            in_names=tuple(in_names),
            out_names=tuple(out_names),
            lowering_input_output_aliases=(),
            sim_require_finite=True,
            sim_require_nnan=True,
            nc=nc,
        )
        return tuple(outs)

    devices = jax.devices()[:NCORES]
    mesh = Mesh(np.asarray(devices), ("core",))
    n_outs = len(out_names)
    in_specs = (PartitionSpec("core"),) * (n_params + n_outs)
    out_specs = (PartitionSpec("core"),) * n_outs
    sharded = jax.jit(
        shard_map(_body, mesh=mesh, in_specs=in_specs, out_specs=out_specs,
                  check_rep=False),
        keep_unused=True,
    )
    per_core = [[np.asarray(m[name]) for name in in_names[:n_params]]
                for m in in_maps]
    sh = NamedSharding(mesh, PartitionSpec("core"))
    concat_in = [
        jax.device_put(
            np.concatenate([per_core[c][i] for c in range(NCORES)], axis=0), sh)
        for i in range(n_params)
    ]
    concat_zeros = [
        jax.device_put(np.zeros((NCORES * z.shape[0], *z.shape[1:]), z.dtype), sh)
        for z in zero_outs
    ]
    jax.block_until_ready(concat_in)
    jax.block_until_ready(concat_zeros)

    times = []
    out_arrs = None
    for i in range(max(2, n_runs)):
        t0 = _time.perf_counter()
        out_arrs = sharded(*concat_in, *concat_zeros)
        jax.block_until_ready(out_arrs)
        times.append(_time.perf_counter() - t0)
    LAST_TIMES = times
    LAST_TIME_NS = int(min(times[1:]) * 1e9)
    return [
        {name: np.asarray(out_arrs[i]).reshape(NCORES, *out_avals[i].shape)[c]
         for i, name in enumerate(out_names)}
        for c in range(NCORES)
    ]
